# revision 20
# baseline (speedup 1.0000x reference)
"""CRF negative-log-likelihood loss on 8 Trainium2 NeuronCores (Bass/Tile).

Problem: nn_CRF — logits [2048, 512, 32], y_ent [2048, 512], lens [2048],
transitions [32, 32] -> per-sequence NLL [2048] = logZ - gold_path_score.

Strategy (data parallel over batch, 256 sequences/core, length-sorted):

  logZ via the forward algorithm, reformulated in the *scaled probability
  domain* so each scan step is one tiny matmul + one elementwise multiply:

      u_{s+1} = W_s  (*)  (E^T u_s)          (fwd)
      g_{s-1} = W_{s-1} (*) (That g_s)       (bwd, in "gamma" form)

  with E = exp(clip(transitions, -32 ln2)) held as stationary block-diagonal
  PE weights and W = exp(logits - rowmax - C) streamed from HBM in bf16.
  All per-(b,t) scale factors are folded into W on the host and undone by
  per-sequence constants at the end.  Steps past a sequence's length are
  made exact no-ops by a one-hot END emission boosted by 2^32 (cancelling
  the 2^-32 clipped END->END transition in bf16), so the backward state at
  t = len is analytically BOOST*e_END and the meet point is free to move.

  The scan is latency-bound: each serial step costs a full PE->PSUM->DVE->
  SBUF round trip (~480-580ns depending on tile width).  Two levers:
   * lens ~ U[1,512], so most sequences need far fewer than 256 steps:
     sequences are length-sorted (stratified over cores) and packed into
     TWO lanes: a NARROW lane (F1 cols) holding the longest sequences
     (256 serial steps at a small per-step latency) and a WIDE lane
     (F2 cols) holding the rest, which meets in the middle of an effective
     length 2*s2 = max len of its sequences (~200 steps).
   * narrow tiles shorten both the matmul and the DVE multiply on the
     critical path (DVE tensor_tensor = (151+F)/0.96 ns).

Layout per core, per lane (F columns): state tile [128 part, F free]:
  partition p = 32*g + tag, g = 2*dir + half, free col = b within half.
  One [128,128] block-diag matmul per lane per step + one DVE multiply.
  Lane 1 holds the 2*F1 longest sequences, lane 2 the 2*F2 others.
"""

import math
import sys

for _p in ("/opt/trn_rl_repo", "/opt/pypackages"):
    if _p not in sys.path:
        sys.path.append(_p)

import numpy as np
import ml_dtypes

BF16 = ml_dtypes.bfloat16
F32 = np.float32

B, T, K = 2048, 512, 32
NCORES = 8
BS = B // NCORES            # 256 sequences per core
NS = T // 2                 # lane-1 serial steps (fwd+bwd meet in middle)
F1 = 28                     # lane-1 width (2*F1 longest seqs per core)
F2 = 100                    # lane-2 width (2*F2 remaining seqs per core)
START_IDX, END_IDX = 0, 1
CLIP = float(32.0 * math.log(2.0))   # forbidden-transition clip; exp = 2^-32 exact in bf16
BOOST = float(2.0 ** 32)
TERMS_F = 1032              # 512 e-terms + 513 t-terms + 7 zero pad

TRACE = False               # test.py sets True to capture an NTFF profile
LAST_RESULTS = None         # BassKernelResults of the last run (for test.py)

_CACHE = {}


def _round_chunks(s2):
    """Chunk schedule over scan rounds: small first chunks so the scan can
    start as soon as possible; returns [(round_lo, round_hi), ...]."""
    sizes = [1, 1, 2, 4, 8, 16] + [32] * 7
    assert sum(sizes) == NS
    out, r0 = [], 0
    for cs in sizes:
        out.append((r0, r0 + cs))
        r0 += cs
    return out


def _col_layout(s2):
    """Flat W-buffer column offset for each (round, lane) slice, in the
    exact order the scan consumes them."""
    offs = {}
    col = 0
    for s in range(NS):
        offs[(s, 0)] = col
        col += F1
        if s < s2:
            offs[(s, 1)] = col
            col += F2
    return offs, col


def _build_program(s2):
    """Build + compile the Bass/Tile program once per (s2)."""
    key = ("nc", s2)
    if key in _CACHE:
        return _CACHE[key]
    import concourse.bacc as bacc
    import concourse.tile as tile
    from concourse import mybir

    nc = bacc.Bacc("TRN2", target_bir_lowering=False, debug=False,
                   enable_asserts=False)
    bf = mybir.dt.bfloat16
    f32 = mybir.dt.float32

    offs, ncols = _col_layout(s2)
    chunks = _round_chunks(s2)
    # chunk column ranges
    cbounds = []
    for (rlo, rhi) in chunks:
        clo = offs[(rlo, 0)]
        last = (rhi - 1, 1) if (rhi - 1) < s2 else (rhi - 1, 0)
        chi = offs[last] + (F2 if last[1] == 1 else F1)
        cbounds.append((clo, chi))

    wdev = nc.dram_tensor("wdev", [128, ncols], bf, kind="ExternalInput")
    # wmm = stationary block-diag scan weights (the only scan-critical DMA)
    wmm = nc.dram_tensor("wmm", [128, 128], bf, kind="ExternalInput")
    winit = nc.dram_tensor("winit", [128, 128], bf, kind="ExternalInput")
    wfin = nc.dram_tensor("wfin", [128, 64], bf, kind="ExternalInput")
    ones2 = nc.dram_tensor("ones2", [64, 2], f32, kind="ExternalInput")
    terms = nc.dram_tensor("terms", [2, 128, TERMS_F], f32,
                           kind="ExternalInput")
    out_logz = nc.dram_tensor("out_logz", [2, 128], f32,
                              kind="ExternalOutput")
    out_score = nc.dram_tensor("out_score", [2, 128, 1], f32,
                               kind="ExternalOutput")

    FS = (F1, F2)
    SS = (NS, s2)

    with tile.TileContext(nc) as tc:
        with (
            tc.tile_pool(name="const", bufs=1) as constp,
            tc.tile_pool(name="wstream", bufs=1) as wp,
            tc.tile_pool(name="state", bufs=NS) as stp,
            tc.tile_pool(name="fin", bufs=1) as finp,
            tc.tile_pool(name="psA", bufs=3, space="PSUM") as psA,
            tc.tile_pool(name="psB", bufs=2, space="PSUM") as psB,
        ):
            # preload the Ln activation table on the idle ACT engine now, so
            # the tail's Ln doesn't pay the ~2.7us ACT_TABLE_LOAD; the Copy
            # accums below reuse the same table set (Copy is in every set)
            lnsrc = constp.tile([64, 1], f32, tag="lnsrc")
            nc.vector.memzero(lnsrc[:])
            lnwarm = finp.tile([64, 1], f32, tag="lnwarm")
            nc.scalar.activation(out=lnwarm[:], in_=lnsrc[:],
                                 func=mybir.ActivationFunctionType.Ln,
                                 bias=1.0)

            # scan-critical stationary weights: one small sync-ring DMA;
            # the (column-constant) initial states ride the scalar ring in
            # parallel, so neither serializes behind the other
            wmm_t = constp.tile([128, 128], bf, tag="wmm_t")
            nc.sync.dma_start(out=wmm_t[:], in_=wmm[:])
            init_t = constp.tile([128, 128], bf, tag="init_t")
            nc.scalar.dma_start(out=init_t[:], in_=winit[:])
            # first two W chunks fetch on the scalar HWDGE ring, concurrent
            # with wmm on the sync ring, so the first TTs aren't starved.
            # All W chunks stay resident (written once, never reused).
            wts = {}
            for ci in (0, 1):
                clo, chi = cbounds[ci]
                wts[ci] = wp.tile([128, chi - clo], bf, tag=f"wt{ci}",
                                  name=f"wt{ci}")
                nc.scalar.dma_start(out=wts[ci][:], in_=wdev[:, clo:chi])
            # finalization-only constants ride behind the critical fetches
            wfin_t = constp.tile([128, 64], bf, tag="wfin_t")
            nc.scalar.dma_start(out=wfin_t[:], in_=wfin[:])
            ones_t = constp.tile([64, 2], f32)
            nc.scalar.dma_start(out=ones_t[:], in_=ones2[:])

            # gold-path score terms: fetched on the sync ring AFTER the
            # early W chunks (ring is FIFO -> cannot crowd startup DMAs)
            terms_t = []
            dump = constp.tile([128, TERMS_F], f32, tag="dump")
            for ch in range(2):
                tt = constp.tile([128, TERMS_F], f32, tag=f"terms{ch}",
                                 name=f"terms{ch}")
                terms_t.append(tt)

            logz_all = finp.tile([2, 128], f32, tag="logz_all")
            state = [init_t[:, 0:F1], init_t[:, F1:F1 + F2]]

            def finalize(g):
                """Z per column of lane g -> logz_all[:, lane cols]."""
                Fg = FS[g]
                c0 = 0 if g == 0 else F1
                beta = psB.tile([64, Fg], f32, tag="meet", name=f"beta{g}")
                nc.tensor.matmul(out=beta[:], lhsT=wfin_t[:],
                                 rhs=state[g][:], start=True, stop=True)
                prod = finp.tile([64, Fg], f32, tag=f"prod{g}",
                                 name=f"prod{g}")
                nc.vector.tensor_tensor(out=prod[:], in0=beta[:],
                                        in1=state[g][0:64, :],
                                        op=mybir.AluOpType.mult)
                z2 = psB.tile([2, Fg], f32, tag="meet", name=f"z2{g}")
                nc.tensor.matmul(out=z2[:], lhsT=ones_t[:], rhs=prod[:],
                                 start=True, stop=True)
                # device Ln is only accurate for inputs in [2^-64, 2^64);
                # fold a 2^-32 prescale in (compensated in host constant HC)
                nc.scalar.activation(out=logz_all[:, c0:c0 + Fg], in_=z2[:],
                                     func=mybir.ActivationFunctionType.Ln,
                                     scale=float(2.0 ** -32))

            wt = None
            for ci, ((rlo, rhi), (clo, chi)) in enumerate(zip(chunks,
                                                              cbounds)):
                if ci in wts:
                    wt = wts[ci]
                else:
                    wt = wp.tile([128, chi - clo], bf, tag=f"wt{ci}",
                                 name=f"wt{ci}")
                    nc.sync.dma_start(out=wt[:], in_=wdev[:, clo:chi])
                if ci == 5:
                    # W stream is far enough ahead; queue the score terms
                    for ch in range(2):
                        nc.sync.dma_start(out=terms_t[ch][:],
                                          in_=terms[ch, :, :])
                    for ch in range(2):
                        sc = finp.tile([128, 1], f32, tag=f"sc{ch}",
                                       name=f"sc{ch}")
                        nc.scalar.activation(
                            out=dump[:], in_=terms_t[ch][:],
                            func=mybir.ActivationFunctionType.Copy,
                            accum_out=sc[:])
                        # score is ready mid-scan; ship it out then
                        nc.scalar.dma_start(out=out_score[ch, :, :],
                                            in_=sc[:])
                for s in range(rlo, rhi):
                    for g in range(2):
                        if s >= SS[g]:
                            continue
                        Fg = FS[g]
                        off = offs[(s, g)] - clo
                        v = psA.tile([128, Fg], f32, tag=f"v{g}",
                                     name=f"v{g}")
                        nc.tensor.matmul(out=v[:], lhsT=wmm_t[:],
                                         rhs=state[g][:],
                                         start=True, stop=True)
                        ns_ = stp.tile([128, Fg], bf, tag=f"st{g}",
                                       name=f"st{g}")
                        nc.vector.tensor_tensor(
                            out=ns_[:], in0=v[:], in1=wt[:, off:off + Fg],
                            op=mybir.AluOpType.mult)
                        state[g] = ns_
                    if s == s2 - 1:
                        finalize(1)     # wide lane retires early
            finalize(0)
            nc.sync.dma_start(out=out_logz[:, :], in_=logz_all[:])

    nc.compile()
    _CACHE[key] = nc
    return nc


def _calibrate_C(logits, lens_, M, E):
    """Mean per-step growth of the scaled forward recursion, estimated on a
    small subsample.  C only conditions dynamic range, never correctness."""
    bs = np.arange(0, B, max(1, B // 128))
    E64 = E.astype(np.float64)
    lg = logits[bs].astype(np.float64)
    Ms = M[bs].astype(np.float64)
    lv = lens_[bs]
    up = np.zeros((K, len(bs))); up[START_IDX] = 1.0
    grs = []
    for t in range(NS):
        up = (E64.T @ up) * np.exp(lg[:, t, :] - Ms[:, t, None]).T
        m = up.max(axis=0)
        live = t < lv
        if live.any():
            grs.append(np.log(m[live]))
        up /= m
        up[:, ~live] = 0.0
        up[START_IDX, ~live] = 1.0
    return float(np.concatenate(grs).mean())


def kernel(logits, y_ent, lens, transitions):
    logits = np.ascontiguousarray(np.asarray(logits), dtype=F32)
    y = np.asarray(y_ent).astype(np.int64)
    lens_ = np.asarray(lens).astype(np.int64)
    trans = np.asarray(transitions).astype(F32)
    assert logits.shape == (B, T, K)

    # ---------------- host preprocessing ----------------
    Tc = np.maximum(trans, F32(-CLIP))
    E = np.exp(Tc.astype(np.float64)).astype(F32)
    E_bf = E.astype(BF16)
    M = logits.max(axis=2)                      # [B, T]
    C = _calibrate_C(logits, lens_, M, E)

    # global length-sort, stratified over cores: sorted position p ->
    # core p%8, within-core rank p//8.  Lane 1 = ranks < 2*F1 (longest),
    # lane 2 = the rest; lane 2 meets in the middle of 2*s2 >= its max len.
    order = np.argsort(-lens_, kind="stable")
    s2 = int(min(NS, max(8, (int(lens_[order[2 * F1 * NCORES]]) + 1) // 2 + 1)))

    # scaled emissions W[t, j, b] in the ORIGINAL batch order
    Wb = np.empty((T, K, B), dtype=BF16)
    pad_TB = np.arange(T)[:, None] >= lens_[None, :]          # [T, B]
    for t0 in range(0, T, 32):
        te = t0 + 32
        w = np.exp(logits[:, t0:te, :] - M[:, t0:te, None] - F32(C))
        w = w.transpose(1, 2, 0)                              # [32, K, B] f32
        pm = pad_TB[t0:te]
        w = np.where(pm[:, None, :], F32(0.0), w)
        w[:, END_IDX, :] = np.where(pm, F32(BOOST), w[:, END_IDX, :])
        Wb[t0:te] = w.astype(BF16)

    # per-core flat W stream in exact consumption order
    offs, ncols = _col_layout(s2)
    FS, SS = (F1, F2), (NS, s2)
    wdev_np = np.empty((NCORES, 128, ncols), dtype=BF16)
    for core in range(NCORES):
        ranks = np.arange(BS) * NCORES + core
        sid = order[ranks]                       # seq ids by within-core rank
        lane_seqs = [
            (sid[0:F1], sid[F1:2 * F1]),                       # lane 1 halves
            (sid[2 * F1:2 * F1 + F2], sid[2 * F1 + F2:BS]),    # lane 2 halves
        ]
        for g in range(2):
            Fg, sg = FS[g], SS[g]
            h0, h1 = lane_seqs[g]
            tf = np.arange(sg)                   # fwd slice at step s
            tb = 2 * sg - 1 - tf                 # bwd slice at step s
            blk = np.empty((sg, 128, Fg), dtype=BF16)
            blk[:, 0:32, :] = Wb[tf][:, :, h0]
            blk[:, 32:64, :] = Wb[tf][:, :, h1]
            blk[:, 64:96, :] = Wb[tb][:, :, h0]
            blk[:, 96:128, :] = Wb[tb][:, :, h1]
            for s in range(sg):
                o = offs[(s, g)]
                wdev_np[core, :, o:o + Fg] = blk[s]

    # stationary scan weights: block-diag(E, E, E^T, E^T)
    wmm_np = np.zeros((128, 128), dtype=BF16)
    wmm_np[0:32, 0:32] = E_bf
    wmm_np[32:64, 32:64] = E_bf
    wmm_np[64:96, 64:96] = E_bf.T
    wmm_np[96:128, 96:128] = E_bf.T
    winit_np = np.zeros((128, 128), dtype=BF16)
    winit_np[0, :] = 1.0                 # fwd one-hot START per block
    winit_np[32, :] = 1.0
    winit_np[64 + END_IDX, :] = BOOST    # bwd gamma init
    winit_np[96 + END_IDX, :] = BOOST
    wfin_np = np.zeros((128, 64), dtype=BF16)
    wfin_np[64:96, 0:32] = E_bf.T        # wfin: beta = That gamma
    wfin_np[96:128, 32:64] = E_bf.T
    ones_np = np.zeros((64, 2), dtype=F32)
    ones_np[0:32, 0] = 1.0
    ones_np[32:64, 1] = 1.0

    # gold-path score terms (host gathers + masks; device sums), in
    # SORTED order so the device layout matches the logz layout
    e_scr = np.take_along_axis(logits, y[:, :, None], axis=2)[:, :, 0]
    e_terms = np.where(np.arange(T)[None, :] < lens_[:, None],
                       e_scr, F32(0.0)).astype(F32)            # [B, 512]
    labels_ext = np.concatenate(
        [np.full((B, 1), START_IDX, np.int64), y,
         np.full((B, 1), END_IDX, np.int64)], axis=1)
    pos = np.arange(T + 2)[None, :]
    labels_ext = np.where(pos < (lens_ + 1)[:, None], labels_ext, END_IDX)
    trn_scr = trans[labels_ext[:, :-1], labels_ext[:, 1:]]
    t_terms = np.where(np.arange(T + 1)[None, :] < (lens_ + 1)[:, None],
                       trn_scr, F32(0.0)).astype(F32)          # [B, 513]
    terms_np = np.zeros((NCORES, 2, 128, TERMS_F), dtype=F32)
    for core in range(NCORES):
        sid = order[np.arange(BS) * NCORES + core]
        terms_np[core, :, :, 0:T] = e_terms[sid].reshape(2, 128, T)
        terms_np[core, :, :, T:2 * T + 1] = t_terms[sid].reshape(2, 128,
                                                                 T + 1)

    # per-sequence constant: logZ = ln(Z_dev * 2^-32) + sum_{t<len}(M+C)
    emask = np.arange(T)[None, :] < lens_[:, None]
    HC = ((M.astype(np.float64) * emask).sum(axis=1)
          + C * lens_).astype(F32)

    # ---------------- run on the 8 cores ----------------
    nc = _build_program(s2)
    from concourse.bass_utils import run_bass_kernel_spmd

    in_maps = [
        dict(wdev=wdev_np[core], wmm=wmm_np, winit=winit_np, wfin=wfin_np,
             ones2=ones_np, terms=terms_np[core])
        for core in range(NCORES)
    ]
    res = run_bass_kernel_spmd(nc, in_maps, core_ids=list(range(NCORES)),
                               trace=TRACE)
    global LAST_RESULTS
    LAST_RESULTS = res

    # decode: logz_all [2, 128] is [half, lane cols]; rank mapping per lane
    nll = np.empty(B, dtype=F32)
    for core in range(NCORES):
        sid = order[np.arange(BS) * NCORES + core]
        lz = res.results[core]["out_logz"]                 # [2, 128]
        score = res.results[core]["out_score"].reshape(BS)  # by rank
        logz_rank = np.empty(BS, dtype=F32)
        logz_rank[0:F1] = lz[0, 0:F1]
        logz_rank[F1:2 * F1] = lz[1, 0:F1]
        logz_rank[2 * F1:2 * F1 + F2] = lz[0, F1:F1 + F2]
        logz_rank[2 * F1 + F2:BS] = lz[1, F1:F1 + F2]
        nll[sid] = logz_rank + HC[sid] - score

    return nll.astype(F32)


# revision 21
# speedup vs baseline: 1.0696x; 1.0696x over previous
"""CRF negative-log-likelihood loss on 8 Trainium2 NeuronCores (Bass/Tile).

Problem: nn_CRF — logits [2048, 512, 32], y_ent [2048, 512], lens [2048],
transitions [32, 32] -> per-sequence NLL [2048] = logZ - gold_path_score.

Strategy (pure data parallel over batch, 256 sequences/core):

  logZ via the forward algorithm, reformulated in the *scaled probability
  domain* so each scan step is one tiny matmul + one elementwise multiply:

      u_{t+1} = W_t  (*)  (E^T u_t)          (fwd)
      g_{t-1} = W_{t-1} (*) (That g_t)       (bwd, in "gamma" form)

  with E = exp(clip(transitions, -32 ln2)) held as stationary block-diagonal
  PE weights and W = exp(logits - rowmax - C) streamed from HBM in bf16.
  All per-(b,t) scale factors (rowmax M, global constant C, pad-step 2^32
  boosts) are folded into W on the host and undone by per-sequence constants
  at the end, so the device scan has zero rescaling ops on the serial path.
  Sequences shorter than T are padded with a one-hot END emission boosted by
  2^32 (exactly cancelling the 2^-32 clipped END->END transition in bf16),
  which makes every padded step an exact no-op and every sequence uniform.

  Forward and backward halves run in the same [128, 64] tiles (4 x 32-tag
  partition blocks: fwd b-half0, fwd b-half1, bwd b-half0, bwd b-half1) and
  meet in the middle after 256 serial steps: Z = sum_j alpha_256[j]*beta_256[j].

  The gold path score is an indexed sum: the host prepares the gathered
  (pre-masked) per-step terms, the device reduces them in f32.

Layout per core, per chain ch in {0,1} (chain = 128 consecutive sequences):
  state tile [128 part, 64 free]: partition p = 32*g + tag, g = 2*dir + half,
  free col = b within half.  One [128,128] block-diag matmul per chain per
  step + one DVE multiply; the two chains pipeline PE against DVE.
"""

import math
import sys

for _p in ("/opt/trn_rl_repo", "/opt/pypackages"):
    if _p not in sys.path:
        sys.path.append(_p)

import numpy as np
import ml_dtypes

BF16 = ml_dtypes.bfloat16
F32 = np.float32

B, T, K = 2048, 512, 32
NCORES = 8
BS = B // NCORES            # 256 sequences per core
NS = T // 2                 # 256 serial scan steps (fwd+bwd meet in middle)
CHUNK = 32                  # scan steps per W DMA chunk
NCHUNK = NS // CHUNK
START_IDX, END_IDX = 0, 1
CLIP = float(32.0 * math.log(2.0))   # forbidden-transition clip; exp = 2^-32 exact in bf16
BOOST = float(2.0 ** 32)
TERMS_F = 1032              # 512 e-terms + 513 t-terms + 7 zero pad

TRACE = False               # test.py sets True to capture an NTFF profile
LAST_RESULTS = None         # BassKernelResults of the last run (for test.py)
DEBUG_OUTPUTS = False       # adds raw-Z/state dumps (debugging only)
TILE_MM = False             # scan matmuls as 4 concurrent 32x32 PE tiles
                            # (measured WORSE: 4 sem-incs add ~100ns/step)

_CACHE = {}


def _build_program():
    """Build + compile the Bass/Tile program once per process."""
    if "nc" in _CACHE:
        return _CACHE["nc"]
    import concourse.bacc as bacc
    import concourse.tile as tile
    from concourse import mybir

    nc = bacc.Bacc("TRN2", target_bir_lowering=False, debug=False,
                   enable_asserts=False)
    bf = mybir.dt.bfloat16
    f32 = mybir.dt.float32

    wdev = nc.dram_tensor("wdev", [128, NS, 2, 64], bf,
                          kind="ExternalInput")
    # cpack = [wmm | winit] scan-critical constants; wfin (finalization
    # weights) ships separately so the startup DMA is as small as possible
    cpack = nc.dram_tensor("cpack", [128, 192], bf, kind="ExternalInput")
    wfin = nc.dram_tensor("wfin", [128, 64], bf, kind="ExternalInput")
    ones2 = nc.dram_tensor("ones2", [64, 2], f32, kind="ExternalInput")
    terms = nc.dram_tensor("terms", [2, 128, TERMS_F], f32,
                           kind="ExternalInput")
    out_logz = nc.dram_tensor("out_logz", [2, 2, 64], f32,
                              kind="ExternalOutput")
    out_score = nc.dram_tensor("out_score", [2, 128, 1], f32,
                               kind="ExternalOutput")
    if DEBUG_OUTPUTS:
        out_z2 = nc.dram_tensor("out_z2", [2, 2, 64], f32,
                                kind="ExternalOutput")
        out_state = nc.dram_tensor("out_state", [2, 128, 64], bf,
                                   kind="ExternalOutput")
        out_prod = nc.dram_tensor("out_prod", [2, 64, 64], f32,
                                  kind="ExternalOutput")

    with tile.TileContext(nc) as tc:
        with (
            tc.tile_pool(name="const", bufs=1) as constp,
            tc.tile_pool(name="wstream", bufs=1) as wp,
            tc.tile_pool(name="state", bufs=NS) as stp,
            tc.tile_pool(name="fin", bufs=1) as finp,
            tc.tile_pool(name="psA", bufs=3, space="PSUM") as psA,
            tc.tile_pool(name="psB", bufs=2, space="PSUM") as psB,
        ):
            # W stream: small first chunks so the scan starts early
            sizes = [1, 1, 2, 4, 8, 16] + [CHUNK] * (NCHUNK - 1)
            assert sum(sizes) == NS
            bounds = []
            s0 = 0
            for cs in sizes:
                bounds.append((s0, cs))
                s0 += cs

            # preload the Ln activation table on the idle ACT engine now, so
            # the tail's Ln doesn't pay the ~2.7us ACT_TABLE_LOAD; the Copy
            # accums below reuse the same table set (Copy is in every set)
            lnsrc = constp.tile([64, 1], f32, tag="lnsrc")
            nc.vector.memzero(lnsrc[:])
            lnwarm = finp.tile([64, 1], f32, tag="lnwarm")
            nc.scalar.activation(out=lnwarm[:], in_=lnsrc[:],
                                 func=mybir.ActivationFunctionType.Ln,
                                 bias=1.0)

            # scan-critical consts land in ONE small sync-queue DMA (one
            # issue + one HBM completion on the startup critical path)
            cp_t = constp.tile([128, 192], bf)
            nc.sync.dma_start(out=cp_t[:], in_=cpack[:])
            wmm_t = cp_t[:, 0:128]
            init_t = cp_t[:, 128:192]
            # first two W chunks fetch on the scalar HWDGE ring, concurrent
            # with cpack on the sync ring, so the first TTs aren't starved
            # all W chunks stay resident (written once, never reused), so
            # Tile emits no write-after-read tracking on the scan's TTs
            wts = {}
            for ci in (0, 1):
                s0, cs = bounds[ci]
                wts[ci] = wp.tile([128, cs, 2, 64], bf, tag=f"wt{ci}",
                                  name=f"wt{ci}")
                nc.scalar.dma_start(out=wts[ci][:],
                                    in_=wdev[:, s0:s0 + cs, :, :])
            # finalization-only constants ride behind the critical fetches
            wfin_t = constp.tile([128, 64], bf, tag="wfin_t")
            nc.scalar.dma_start(out=wfin_t[:], in_=wfin[:])
            ones_t = constp.tile([64, 2], f32)
            nc.scalar.dma_start(out=ones_t[:], in_=ones2[:])

            # gold-path score terms: fetched on the sync ring AFTER the first
            # five W chunks (the ring is FIFO, so this 1MB fetch cannot crowd
            # the scan-critical startup DMAs); reduced on the idle ACT engine
            terms_t = []
            dump = constp.tile([128, TERMS_F], f32, tag="dump")
            for ch in range(2):
                tt = constp.tile([128, TERMS_F], f32, tag=f"terms{ch}")
                terms_t.append(tt)

            state = [init_t, init_t]
            for ci, (s0, cs) in enumerate(bounds):
                if ci in wts:
                    wt = wts[ci]
                else:
                    wt = wp.tile([128, cs, 2, 64], bf, tag=f"wt{ci}")
                    nc.sync.dma_start(out=wt[:],
                                      in_=wdev[:, s0:s0 + cs, :, :])
                if ci == 5:
                    # W stream is far enough ahead; queue the score terms now
                    for ch in range(2):
                        nc.sync.dma_start(out=terms_t[ch][:],
                                          in_=terms[ch, :, :])
                    for ch in range(2):
                        sc = finp.tile([128, 1], f32, tag=f"sc{ch}")
                        nc.scalar.activation(
                            out=dump[:], in_=terms_t[ch][:],
                            func=mybir.ActivationFunctionType.Copy,
                            accum_out=sc[:])
                        # score is ready mid-scan; ship it out then
                        nc.scalar.dma_start(out=out_score[ch, :, :],
                                            in_=sc[:])
                for s in range(cs):
                    for ch in range(2):
                        v = psA.tile([128, 64], f32, tag=f"v{ch}")
                        if TILE_MM:
                            # 4 concurrent 32x32 PE tiles: the block-diagonal
                            # weight means output block g only needs input
                            # block g; a 32-row tile drains the array ~40ns
                            # sooner than the full 128-row path
                            for g in range(4):
                                p0 = 32 * g
                                nc.tensor.matmul(
                                    out=v[p0:p0 + 32, :],
                                    lhsT=wmm_t[p0:p0 + 32, p0:p0 + 32],
                                    rhs=state[ch][p0:p0 + 32, :],
                                    start=True, stop=True,
                                    tile_position=(p0, p0))
                        else:
                            nc.tensor.matmul(out=v[:], lhsT=wmm_t[:],
                                             rhs=state[ch][:],
                                             start=True, stop=True)
                        ns_ = stp.tile([128, 64], bf, tag=f"st{ch}")
                        nc.vector.tensor_tensor(
                            out=ns_[:], in0=v[:], in1=wt[:, s, ch, :],
                            op=mybir.AluOpType.mult)
                        state[ch] = ns_

            logz_all = finp.tile([2, 128], f32, tag="logz_all")
            for ch in range(2):
                # beta_256 = That @ gamma_256 (weights only over bwd rows)
                beta = psB.tile([64, 64], f32, tag="meet")
                nc.tensor.matmul(out=beta[:], lhsT=wfin_t[:],
                                 rhs=state[ch][:], start=True, stop=True)
                # prod = alpha_256 (*) beta_256, tag-aligned on partitions 0-63
                prod = finp.tile([64, 64], f32, tag=f"prod{ch}")
                nc.vector.tensor_tensor(out=prod[:], in0=beta[:],
                                        in1=state[ch][0:64, :],
                                        op=mybir.AluOpType.mult)
                # Z per sequence: sum over each 32-tag block (ones matmul)
                z2 = psB.tile([2, 64], f32, tag="meet")
                nc.tensor.matmul(out=z2[:], lhsT=ones_t[:], rhs=prod[:],
                                 start=True, stop=True)
                # device Ln is only accurate for inputs in [2^-64, 2^64);
                # Z reaches ~2^80, so fold a 2^-32 prescale into the
                # activation (compensated in the host constant HC).
                # layout: [half, ch*64+col] -> one DMA ships both chains
                nc.scalar.activation(out=logz_all[:, 64 * ch:64 * ch + 64],
                                     in_=z2[:],
                                     func=mybir.ActivationFunctionType.Ln,
                                     scale=float(2.0 ** -32))
                if DEBUG_OUTPUTS:
                    z2c = finp.tile([2, 64], f32, tag=f"z2c{ch}")
                    nc.vector.tensor_copy(out=z2c[:], in_=z2[:])
                    nc.sync.dma_start(out=out_z2[ch, :, :], in_=z2c[:])
                    nc.sync.dma_start(out=out_state[ch, :, :], in_=state[ch][:])
                    nc.sync.dma_start(out=out_prod[ch, :, :], in_=prod[:])
            nc.sync.dma_start(out=out_logz[:, :, :], in_=logz_all[:])

    nc.compile()
    _CACHE["nc"] = nc
    return nc


def _calibrate_C(logits, lens_, M, E):
    """Mean per-step growth of the scaled forward recursion, estimated on a
    small subsample.  C only conditions dynamic range, never correctness."""
    bs = np.arange(0, B, max(1, B // 128))
    E64 = E.astype(np.float64)
    lg = logits[bs].astype(np.float64)
    Ms = M[bs].astype(np.float64)
    lv = lens_[bs]
    up = np.zeros((K, len(bs))); up[START_IDX] = 1.0
    grs = []
    for t in range(NS):
        up = (E64.T @ up) * np.exp(lg[:, t, :] - Ms[:, t, None]).T
        m = up.max(axis=0)
        live = t < lv
        if live.any():
            grs.append(np.log(m[live]))
        up /= m
        up[:, ~live] = 0.0
        up[START_IDX, ~live] = 1.0
    return float(np.concatenate(grs).mean())


def kernel(logits, y_ent, lens, transitions):
    logits = np.ascontiguousarray(np.asarray(logits), dtype=F32)
    y = np.asarray(y_ent).astype(np.int64)
    lens_ = np.asarray(lens).astype(np.int64)
    trans = np.asarray(transitions).astype(F32)
    assert logits.shape == (B, T, K)

    # ---------------- host preprocessing ----------------
    Tc = np.maximum(trans, F32(-CLIP))
    E = np.exp(Tc.astype(np.float64)).astype(F32)
    E_bf = E.astype(BF16)
    M = logits.max(axis=2)                      # [B, T]
    C = _calibrate_C(logits, lens_, M, E)

    # scaled emissions W[t, j, b] (slots 0..511; slot 512 is the all-pad init)
    Wb = np.empty((T, K, B), dtype=BF16)
    pad_TB = np.arange(T)[:, None] >= lens_[None, :]          # [T, B]
    for t0 in range(0, T, 32):
        te = t0 + 32
        w = np.exp(logits[:, t0:te, :] - M[:, t0:te, None] - F32(C))
        w = w.transpose(1, 2, 0)                              # [32, K, B] f32
        pm = pad_TB[t0:te]
        w = np.where(pm[:, None, :], F32(0.0), w)
        w[:, END_IDX, :] = np.where(pm, F32(BOOST), w[:, END_IDX, :])
        Wb[t0:te] = w.astype(BF16)

    # pack per-core W stream: [core, p=(dir,half,tag), S, ch, col]
    fwd = Wb[0:NS]                       # serial step s uses slot s
    bwd = Wb[T - 1:NS - 1:-1]            # serial step s uses slot 511-s
    A = np.stack([fwd, bwd], axis=1)     # [S, dir, K, B]
    A = A.reshape(NS, 2, K, NCORES, 2, 2, 64)   # [S, dir, j, core, ch, half, col]
    A = np.ascontiguousarray(A.transpose(3, 1, 5, 2, 0, 4, 6))
    wdev_np = A.reshape(NCORES, 128, NS, 2, 64)

    # constant small tensors: cpack cols 0:128 = wmm, 128:192 = winit;
    # wfin ships separately (finalization only, off the startup path)
    cpack_np = np.zeros((128, 192), dtype=BF16)
    cpack_np[0:32, 0:32] = E_bf          # fwd blocks: lhsT = E
    cpack_np[32:64, 32:64] = E_bf
    cpack_np[64:96, 64:96] = E_bf.T      # bwd blocks: lhsT = E^T
    cpack_np[96:128, 96:128] = E_bf.T
    cpack_np[0, 128:192] = 1.0           # init: fwd one-hot START per block
    cpack_np[32, 128:192] = 1.0
    cpack_np[64 + END_IDX, 128:192] = BOOST  # init: bwd gamma_512
    cpack_np[96 + END_IDX, 128:192] = BOOST
    wfin_np = np.zeros((128, 64), dtype=BF16)
    wfin_np[64:96, 0:32] = E_bf.T        # wfin: beta = That gamma
    wfin_np[96:128, 32:64] = E_bf.T

    ones_np = np.zeros((64, 2), dtype=F32)
    ones_np[0:32, 0] = 1.0
    ones_np[32:64, 1] = 1.0

    # gold-path score terms (host gathers + masks; device sums)
    e_scr = np.take_along_axis(logits, y[:, :, None], axis=2)[:, :, 0]
    e_terms = np.where(np.arange(T)[None, :] < lens_[:, None],
                       e_scr, F32(0.0)).astype(F32)            # [B, 512]
    labels_ext = np.concatenate(
        [np.full((B, 1), START_IDX, np.int64), y,
         np.full((B, 1), END_IDX, np.int64)], axis=1)
    pos = np.arange(T + 2)[None, :]
    labels_ext = np.where(pos < (lens_ + 1)[:, None], labels_ext, END_IDX)
    trn_scr = trans[labels_ext[:, :-1], labels_ext[:, 1:]]
    t_terms = np.where(np.arange(T + 1)[None, :] < (lens_ + 1)[:, None],
                       trn_scr, F32(0.0)).astype(F32)          # [B, 513]
    terms_np = np.zeros((NCORES, 2, 128, TERMS_F), dtype=F32)
    terms_np[..., 0:T] = e_terms.reshape(NCORES, 2, 128, T)
    terms_np[..., T:2 * T + 1] = t_terms.reshape(NCORES, 2, 128, T + 1)

    # per-sequence constant: logZ = ln(Z_dev * 2^-32) + sum_{t<len}(M+C)
    # (- 32 ln2 chain correction + 32 ln2 Ln-prescale compensation cancel)
    emask = np.arange(T)[None, :] < lens_[:, None]
    HC = ((M.astype(np.float64) * emask).sum(axis=1)
          + C * lens_).astype(F32)

    # ---------------- run on the 8 cores ----------------
    nc = _build_program()
    from concourse.bass_utils import run_bass_kernel_spmd

    in_maps = [
        dict(wdev=wdev_np[core], cpack=cpack_np, wfin=wfin_np,
             ones2=ones_np, terms=terms_np[core])
        for core in range(NCORES)
    ]
    res = run_bass_kernel_spmd(nc, in_maps, core_ids=list(range(NCORES)),
                               trace=TRACE)
    global LAST_RESULTS
    LAST_RESULTS = res

    # device logz layout is [half, ch*64+col]; flatten to (ch, half, col)
    logz = np.concatenate(
        [r["out_logz"].transpose(1, 0, 2).reshape(-1)
         for r in res.results]).astype(F32)  # [B]
    score = np.concatenate(
        [r["out_score"].reshape(-1) for r in res.results]).astype(F32)

    return (logz + HC - score).astype(F32)

